# revision 29
# baseline (speedup 1.0000x reference)
"""AKConv + LKA fused Trainium2 kernel, batch-parallel across 8 NeuronCores.

Per-core (1 image, C=256, 64x64), chunk-pipelined front end: per half-image
chunk the offset conv (PE, tap-outer over 4 PSUM banks) feeds bilinear index
math (DVE, [128,16,3] slices) and the PE-transpose fold into the gather's
16-partition-wrapped int16 index layout; dma_gather rounds then stream per
(half, n) with 3 gather buffers in flight, GPSIMD issuing only gathers so the
Pool sequencer never convoys behind combine work.  The bilinear combine runs
3 DVE stt chains + 1 ACT product group (GPSIMD merge) per half, each engine
owning its own accumulator tile.  dma-transpose to channel-major -> ak conv
(PE, lagging one half) -> BN batch stats (+AllReduce) -> fused BN+SiLU (ACT)
-> 5x5 dw + 7x7 dil3 dw with taps split PE (diag matmuls, tap-outer over 4
PSUM banks) / DVE (4x-mode products + 2x adds at [128,2048]) / ACT (products,
DVE adds); DVE partials fold into PSUM via identity matmuls -> per-ct 1x1
conv + gate (GPSIMD mult) -> final 1x1 conv + residual (PSUM-fused) -> out.

PE p-state: the tensor engine drops to ~1/3.7 speed after any idle gap and
needs ~3us of continuous work to re-reach 2.4GHz, so dependency-free
keep-warm matmuls are interleaved wherever PE would otherwise idle.

DVE op-form rules (cost model): tensor_scalar = 4x mode, tensor_tensor = 2x,
scalar_tensor_tensor = 1x; TensorScalarPtr is NOT legal on Pool/GPSIMD.
"""

import numpy as np
import ml_dtypes

import concourse.bass as bass
import concourse.bacc as bacc
import concourse.mybir as mybir
from concourse import tile
from concourse.bass_utils import run_bass_kernel_spmd
from concourse.library_config import mlp

BF16 = mybir.dt.bfloat16
F32 = mybir.dt.float32
I16 = mybir.dt.int16
I32 = mybir.dt.int32
AF = mybir.ActivationFunctionType
OP = mybir.AluOpType
AX = mybir.AxisListType

B, C, H, W = 8, 256, 64, 64
HW = H * W
NBLK = 32
EPS = 1e-5
P1 = 66   # pad-1 layout (offset conv input, residual source)
P2 = 68   # pad-2 layout (y_act, 5x5 dw input)
P9 = 82   # pad-9 layout (z1, 7x7 dil-3 dw input)

# depthwise tap assignment: [0, PE) PE diag matmuls, then DVE product+add
# chains, then ACT products (DVE adds).
PE5, DV5, AC5 = 14, 7, 4     # 25 taps
PE7, DV7, AC7 = 31, 11, 7    # 49 taps


def _win(t, off, rstride, nrows=8, ncols=64):
    """[128, nrows, ncols] window AP into padded tile t at element offset off."""
    a = t[:] if not isinstance(t, bass.AP) else t
    return bass.AP(a.tensor, a.offset + off, [a.ap[0], [rstride, nrows], [1, ncols]])


def build_graph(n_cores: int):
    nc = bacc.Bacc(None, target_bir_lowering=False, num_swdge_queues=4)
    cores = list(range(n_cores))
    count = float(n_cores * HW)

    def par(name, shape, dt):
        return nc.declare_dram_parameter(name, shape, dt, isOutput=False)

    xpad = par("xpad", [2, 128, P1 * P1], BF16)
    xpm4 = par("xpm4", [128 * 128, 1024], BF16)
    basex = par("basex", [128, NBLK, 3], F32)
    basey = par("basey", [128, NBLK, 3], F32)
    woff = par("woff", [128, 108], BF16)         # [(k,t)*6] pre-laid
    pb = par("pb", [6, 1], F32)
    wak = par("wak", [128, 12 * 128], BF16)
    gamma = par("gamma", [128, 2], F32)
    beta = par("beta", [128, 2], F32)
    wd5 = par("wd5", [128, PE5 * 2 * 128], BF16)   # PE taps, diag
    wd7 = par("wd7", [128, PE7 * 2 * 128], BF16)   # PE taps, diag
    w5v = par("w5v", [128, 2, DV5 + AC5], F32)     # DVE+ACT tap scalars
    w7v = par("w7v", [128, 2, DV7 + AC7], F32)
    b5 = par("b5", [128, 2], F32)
    b7 = par("b7", [128, 2], F32)
    wlka1 = par("wlka1", [128, 4 * 128], BF16)
    blka1 = par("blka1", [128, 2], F32)
    wconv = par("wconv", [128, 4 * 128], BF16)
    bconv = par("bconv", [128, 2], F32)
    ident = par("ident", [128, 128], BF16)
    ident32 = par("ident32", [128, 128], F32)
    out = nc.declare_dram_parameter("out", [2, 128, HW], F32, isOutput=True)

    if n_cores > 1:
        stats_in = nc.dram_tensor("stats_in", [128, 4], F32)
        stats_out = nc.dram_tensor("stats_out", [128, 4], F32, addr_space="Shared")

    with tile.TileContext(nc) as tc:
        with (
            tc.tile_pool(name="const", bufs=1) as cp,
            tc.tile_pool(name="act", bufs=1) as ap_,
            tc.tile_pool(name="sm", bufs=1) as sp,
            tc.tile_pool(name="wk", bufs=2) as wp,
            tc.tile_pool(name="gt", bufs=1) as gp,
            tc.tile_pool(name="ga", bufs=2) as gacc,
            tc.tile_pool(name="ps", bufs=4, space="PSUM") as pp,
            tc.tile_pool(name="ps6", bufs=2, space="PSUM") as p6,
        ):
            nc.gpsimd.load_library(mlp)

            # ---------------- constant loads (all contiguous) ----------------
            def cload(name, shape, dt, src, tag=None):
                t = cp.tile(shape, dt, name=name, tag=tag or name)
                nc.sync.dma_start(out=t[:], in_=src)
                return t

            xpad_sb = [cload(f"xpad{k}", [128, P1 * P1], BF16, xpad[k])
                       for k in range(2)]
            woff_sb = cload("woffs", [128, 108], BF16, woff[:])
            wak_sb = cload("waks", [128, 12 * 128], BF16, wak[:])
            wlka1_sb = cload("wlka1s", [128, 4 * 128], BF16, wlka1[:])
            wconv_sb = cload("wconvs", [128, 4 * 128], BF16, wconv[:])
            ident_sb = cload("idents", [128, 128], BF16, ident[:])
            id32_sb = cload("id32s", [128, 128], F32, ident32[:])
            basex_sb = cload("basexs", [128, NBLK, 3], F32, basex[:])
            basey_sb = cload("baseys", [128, NBLK, 3], F32, basey[:])
            pb_sb = cload("pbs", [6, 1], F32, pb[:])
            w5v_sb = cload("w5vs", [128, 2, DV5 + AC5], F32, w5v[:])
            w7v_sb = cload("w7vs", [128, 2, DV7 + AC7], F32, w7v[:])
            small_params = {}
            for nm, h in (("gamma", gamma), ("beta", beta), ("b5", b5),
                          ("b7", b7), ("blka1", blka1), ("bconv", bconv)):
                small_params[nm] = cload(nm + "s", [128, 2], F32, h[:])

            # PE keep-warm: dependency-free matmuls into a scratch PSUM bank.
            warm = p6.tile([128, 512], F32, name="warm", tag="warm", bufs=1)

            def warm_pe(k):
                for _ in range(k):
                    nc.tensor.matmul(out=warm[:], lhsT=ident_sb[:],
                                     rhs=xpad_sb[0][:, 0:512],
                                     start=True, stop=True)

            warm_pe(20)

            yact = [ap_.tile([128, P2 * P2], BF16, name=f"yact{m}", tag=f"yact{m}")
                    for m in range(2)]
            for m in range(2):
                # only the pad border needs zeroing; interior is overwritten
                t = yact[m][:]
                nc.gpsimd.memset(bass.AP(t.tensor, t.offset, [t.ap[0], [1, 2 * P2]]), 0.0)
                nc.gpsimd.memset(bass.AP(t.tensor, t.offset + 66 * P2,
                                         [t.ap[0], [1, 2 * P2]]), 0.0)
                nc.gpsimd.memset(bass.AP(t.tensor, t.offset + 2 * P2,
                                         [t.ap[0], [P2, 64], [1, 2]]), 0.0)
                nc.gpsimd.memset(bass.AP(t.tensor, t.offset + 2 * P2 + 66,
                                         [t.ap[0], [P2, 64], [1, 2]]), 0.0)

            # ------- stages 1-4, chunk/half pipelined ------------------------
            xoff = [ap_.tile([128, 2, HW], BF16, name=f"xoff{n}", tag=f"xoff{n}")
                    for n in range(3)]
            ypre = ap_.tile([128, 2, HW], BF16, name="ypre", tag="ypre")
            sump = sp.tile([128, 2, 8], F32, name="sump")
            sqp = sp.tile([128, 2, 8], F32, name="sqp")

            def mkq(tag, bufs=1):
                return sp.tile([128, 16, 3], F32, name=tag, tag=tag, bufs=bufs)

            def idx_side_q(base_ap, off_ap, lim, s):
                p = mkq(f"p{s}")
                nc.vector.tensor_tensor(out=p[:], in0=base_ap, in1=off_ap, op=OP.add)
                t = mkq(f"t{s}")
                nc.vector.tensor_scalar(out=t[:], in0=p[:], scalar1=1024.0,
                                        scalar2=None, op0=OP.add)
                ui = sp.tile([128, 16, 3], I32, name=f"ui{s}", tag=f"ui{s}", bufs=2)
                nc.vector.tensor_copy(out=ui[:], in_=t[:])
                uf = mkq(f"uf{s}")
                nc.vector.tensor_copy(out=uf[:], in_=ui[:])
                nc.vector.tensor_tensor(out=t[:], in0=t[:], in1=uf[:], op=OP.subtract)
                nc.vector.tensor_scalar(out=t[:], in0=t[:], scalar1=0.0,
                                        scalar2=None, op0=OP.is_lt)
                qf = mkq(f"qf{s}")
                nc.vector.scalar_tensor_tensor(out=qf[:], in0=uf[:], scalar=1024.0,
                                               in1=t[:], op0=OP.subtract,
                                               op1=OP.subtract)
                qlt = mkq(f"qlt{s}")
                nc.vector.tensor_scalar(out=qlt[:], in0=qf[:], scalar1=0.0,
                                        scalar2=float(lim), op0=OP.max, op1=OP.min)
                qrb = mkq(f"qrb{s}")
                nc.vector.tensor_scalar(out=qrb[:], in0=qf[:], scalar1=1.0,
                                        scalar2=0.0, op0=OP.add, op1=OP.max)
                nc.vector.tensor_scalar(out=qrb[:], in0=qrb[:], scalar1=float(lim),
                                        scalar2=None, op0=OP.min)
                pc = p
                nc.vector.tensor_scalar(out=pc[:], in0=p[:], scalar1=0.0,
                                        scalar2=float(lim), op0=OP.max, op1=OP.min)
                wlt = mkq(f"wlt{s}")
                nc.vector.scalar_tensor_tensor(out=wlt[:], in0=qlt[:], scalar=1.0,
                                               in1=pc[:], op0=OP.add, op1=OP.subtract)
                wrb = mkq(f"wrb{s}")
                nc.vector.scalar_tensor_tensor(out=wrb[:], in0=pc[:], scalar=1.0,
                                               in1=qrb[:], op0=OP.add, op1=OP.subtract)
                return qlt, qrb, wlt, wrb

            def ak_conv_half(ct):
                for m in range(2):
                    ps = pp.tile([128, 512], F32, name="mm", tag="mm")
                    i = 0
                    for n in range(3):
                        for k in range(2):
                            nc.tensor.matmul(
                                out=ps[:],
                                lhsT=wak_sb[:, (n * 4 + k * 2 + m) * 128:
                                            (n * 4 + k * 2 + m) * 128 + 128],
                                rhs=xoff[n][:, k, ct * 512:(ct + 1) * 512],
                                start=(i == 0), stop=(i == 5))
                            i += 1
                    ysl = ypre[:, m, ct * 512:(ct + 1) * 512]
                    nc.scalar.activation(out=ysl, in_=ps[:], func=AF.Copy,
                                         accum_out=sump[:, m, ct:ct + 1])
                    sq_ps = p6.tile([128, 512], F32, name="sqs", tag="ptr")
                    nc.scalar.activation(out=sq_ps[:], in_=ps[:],
                                         func=AF.Square,
                                         accum_out=sqp[:, m, ct:ct + 1])

            pending_ak = []

            for c in range(2):
                # -- offset conv for cts 4c..4c+3, tap-outer over 4 banks --
                banks = [pp.tile([6, 512], F32, name=f"psoff{cb}", tag="mm")
                         for cb in range(4)]
                i = 0
                for k in range(2):
                    for t in range(9):
                        dy, dx = t // 3, t % 3
                        for cb in range(4):
                            ct = 4 * c + cb
                            nc.tensor.matmul(
                                out=banks[cb][:],
                                lhsT=woff_sb[:, (k * 9 + t) * 6:(k * 9 + t) * 6 + 6],
                                rhs=_win(xpad_sb[k], (ct * 8 + dy) * P1 + dx, P1),
                                start=(i == 0), stop=(i == 17))
                        i += 1
                Tc = sp.tile([128, 16, 6], F32, name="Tc", tag="Tc", bufs=2)
                for cb in range(4):
                    ob = wp.tile([6, 512], F32, name="ob", tag="ob", bufs=2)
                    nc.scalar.activation(out=ob[:], in_=banks[cb][:],
                                         func=AF.Identity, bias=pb_sb[:, 0:1])
                    for j in range(4):
                        pt = p6.tile([128, 6], F32, name="ptr", tag="ptr")
                        nc.tensor.transpose(out=pt[:],
                                            in_=ob[:, j * 128:(j + 1) * 128],
                                            identity=id32_sb[:6, :6])
                        nc.vector.tensor_copy(out=Tc[:, cb * 4 + j, :], in_=pt[:])

                # -- index + bilinear weight math for this chunk --
                bx = basex_sb[:, 16 * c:16 * c + 16, :]
                by = basey_sb[:, 16 * c:16 * c + 16, :]
                qlx, qrx, wxl, wxr = idx_side_q(bx, Tc[:, :, 0:3], H - 1, "x")
                qly, qry, wyl, wyr = idx_side_q(by, Tc[:, :, 3:6], W - 1, "y")
                gwc = []
                for gi, (wx, wy) in enumerate(((wxl, wyl), (wxr, wyr),
                                               (wxl, wyr), (wxr, wyl))):
                    g = mkq(f"g{gi}", bufs=2)
                    nc.vector.tensor_tensor(out=g[:], in0=wx[:], in1=wy[:],
                                            op=OP.mult)
                    gwc.append(g)

                # fold into 16-partition-wrapped int16 index layout
                rx = mkq("rx")
                nc.vector.tensor_tensor(out=rx[:], in0=qlx[:], in1=qrx[:], op=OP.add)
                ry = mkq("ry")
                nc.vector.tensor_tensor(out=ry[:], in0=qly[:], in1=qry[:], op=OP.add)
                uf_t = sp.tile([128, 48], F32, name="uft", tag="uft", bufs=2)
                for n in range(3):
                    nc.vector.scalar_tensor_tensor(
                        out=uf_t[:, n * 16:(n + 1) * 16],
                        in0=rx[:, :, n], scalar=128.0, in1=ry[:, :, n],
                        op0=OP.mult, op1=OP.add)
                pv = p6.tile([48, 128], F32, name="ptr", tag="ptr")
                nc.tensor.transpose(out=pv[:], in_=uf_t[:], identity=id32_sb[:])
                v_t = sp.tile([48, 128], F32, name="vt", tag="vt", bufs=2)
                nc.vector.tensor_copy(out=v_t[:], in_=pv[:])
                wi16 = sp.tile([16, 3, 128], I16, name="wi16", tag="wi16", bufs=2)
                for ph in range(8):
                    pw = p6.tile([16, 48], F32, name="ptr", tag="ptr")
                    nc.tensor.transpose(out=pw[:],
                                        in_=v_t[:, 16 * ph:16 * ph + 16],
                                        identity=id32_sb[:48, :48])
                    for n in range(3):
                        dst = wi16[:, n, :]
                        dst = bass.AP(dst.tensor, dst.offset + ph,
                                      [dst.ap[0], [8, 16]])
                        nc.vector.tensor_copy(out=dst,
                                              in_=pw[:, n * 16:(n + 1) * 16])
                idxw = sp.tile([128, 3, 128], I16, name="idxw", tag="idxw", bufs=2)
                for g8 in range(8):
                    nc.sync.dma_start(out=idxw[16 * g8:16 * g8 + 16, :, :],
                                      in_=wi16[:, :, :])

                # -- gathers (3 buffers in flight) + combine per (half, n) --
                def gather(s):
                    hi, n = s // 3, s % 3
                    j = c * 12 + s
                    g_ = gp.tile([128, 4, 1024], BF16, name="G",
                                 tag=f"G{j % 4}", bufs=1)
                    nc.gpsimd.dma_gather(
                        g_[:], xpm4[:],
                        idxw[:, n, 32 * hi:32 * hi + 32],
                        512, 512, 1024, queue_num=j % 4)
                    return g_

                def combine(g_, s):
                    hi, n = s // 3, s % 3
                    # single-writer acc laid [128, k, b, oc128] so the
                    # channel-major eviction needs only 2 batched transposes
                    acc = gacc.tile([128, 2, 4, 128], BF16, name="acc",
                                    tag="acc", bufs=3)
                    av = acc[:]
                    for jj in range(4):
                        b_ = 4 * hi + jj
                        dst_ = bass.AP(av.tensor, av.offset + jj * 128,
                                       [av.ap[0], [512, 2], [1, 128]])
                        nc.vector.tensor_scalar(
                            out=dst_, in0=g_[:, jj, 0:256],
                            scalar1=gwc[0][:, b_, n:n + 1], scalar2=None,
                            op0=OP.mult)
                        for ci in range(1, 4):
                            src_ = g_[:, jj, ci * 256:(ci + 1) * 256]
                            src_ = bass.AP(src_.tensor, src_.offset,
                                           [src_.ap[0], [128, 2], [1, 128]])
                            nc.vector.scalar_tensor_tensor(
                                out=dst_, in0=src_,
                                scalar=gwc[ci][:, b_, n:n + 1],
                                in1=dst_, op0=OP.mult, op1=OP.add)
                    bg0 = c * 16 + 4 * hi
                    xv = xoff[n][:]
                    for k in range(2):
                        # out as [part, tile, 128] so the xbar writes each
                        # 128x128 tile to its own 128-pixel raster block
                        dst_t = bass.AP(xv.tensor,
                                        xv.offset + k * HW + bg0 * 128,
                                        [xv.ap[0], [128, 4], [1, 128]])
                        nc.sync.dma_start_transpose(
                            out=dst_t, in_=acc[:, k, :, :])

                pend = {}
                for s in range(4):
                    pend[s] = gather(s)
                for s in range(12):
                    if s + 4 < 12:
                        pend[s + 4] = gather(s + 4)
                    combine(pend.pop(s), s)
                    if s % 6 == 5:
                        pending_ak.append(4 * c + 2 * (s // 6))
                        if len(pending_ak) > 1:
                            ct0 = pending_ak.pop(0)
                            ak_conv_half(ct0)
                            ak_conv_half(ct0 + 1)
            for ct0 in pending_ak:
                ak_conv_half(ct0)
                ak_conv_half(ct0 + 1)

            stats_sb = sp.tile([128, 4], F32, name="stats_sb")
            for m in range(2):
                nc.vector.tensor_reduce(out=stats_sb[:, 2 * m:2 * m + 1],
                                        in_=sump[:, m, :], axis=AX.X, op=OP.add)
                nc.vector.tensor_reduce(out=stats_sb[:, 2 * m + 1:2 * m + 2],
                                        in_=sqp[:, m, :], axis=AX.X, op=OP.add)
            if n_cores > 1:
                nc.gpsimd.dma_start(out=stats_in[:], in_=stats_sb[:])
                nc.gpsimd.collective_compute(
                    "AllReduce", OP.add, replica_groups=[cores],
                    ins=[stats_in[:]], outs=[stats_out[:]])

            # stats-independent work issued here so it overlaps the AllReduce
            wd5_sb = ap_.tile([128, PE5 * 2 * 128], BF16, name="wdw", tag="ypre2")
            nc.sync.dma_start(out=wd5_sb[:], in_=wd5[:])
            z1 = [ap_.tile([128, P9 * P9], BF16, name=f"z1{m}", tag=f"xoff{m}")
                  for m in range(2)]
            for m in range(2):
                t = z1[m][:]
                nc.gpsimd.memset(bass.AP(t.tensor, t.offset, [t.ap[0], [1, 9 * P9]]), 0.0)
                nc.gpsimd.memset(bass.AP(t.tensor, t.offset + 73 * P9,
                                         [t.ap[0], [1, 9 * P9]]), 0.0)
                nc.gpsimd.memset(bass.AP(t.tensor, t.offset + 9 * P9,
                                         [t.ap[0], [P9, 64], [1, 9]]), 0.0)
                nc.gpsimd.memset(bass.AP(t.tensor, t.offset + 9 * P9 + 73,
                                         [t.ap[0], [P9, 64], [1, 9]]), 0.0)
            epsc = sp.tile([128, 1], F32, name="epsc")
            nc.vector.memset(epsc[:], EPS)
            warm_pe(45)    # keep PE hot across the AllReduce barrier

            if n_cores > 1:
                stats_all = sp.tile([128, 4], F32, name="stats_all")
                nc.gpsimd.dma_start(out=stats_all[:], in_=stats_out[:])
            else:
                stats_all = stats_sb

            mean = sp.tile([128, 2], F32, name="mean")
            ex2 = sp.tile([128, 2], F32, name="ex2")
            sa = stats_all[:]
            nc.vector.tensor_scalar(
                out=mean[:], in0=bass.AP(sa.tensor, sa.offset, [sa.ap[0], [2, 2]]),
                scalar1=1.0 / count, scalar2=None, op0=OP.mult)
            nc.vector.tensor_scalar(
                out=ex2[:], in0=bass.AP(sa.tensor, sa.offset + 1, [sa.ap[0], [2, 2]]),
                scalar1=1.0 / count, scalar2=None, op0=OP.mult)
            negv = sp.tile([128, 2], F32, name="negv")
            for m in range(2):
                nc.vector.scalar_tensor_tensor(
                    out=negv[:, m:m + 1], in0=mean[:, m:m + 1],
                    scalar=mean[:, m:m + 1], in1=ex2[:, m:m + 1],
                    op0=OP.mult, op1=OP.subtract)
            std = sp.tile([128, 2], F32, name="std")
            nc.scalar.activation(out=std[:], in_=negv[:], func=AF.Sqrt,
                                 bias=epsc[:, 0:1], scale=-1.0)
            inv = sp.tile([128, 2], F32, name="inv")
            nc.vector.reciprocal(out=inv[:], in_=std[:])
            scale = sp.tile([128, 2], F32, name="scale")
            nc.vector.tensor_tensor(out=scale[:], in0=small_params["gamma"][:],
                                    in1=inv[:], op=OP.mult)
            nsc = sp.tile([128, 2], F32, name="nsc")
            nc.vector.tensor_scalar(out=nsc[:], in0=scale[:], scalar1=-1.0,
                                    scalar2=None, op0=OP.mult)
            shift = sp.tile([128, 2], F32, name="shift")
            for m in range(2):
                nc.vector.scalar_tensor_tensor(
                    out=shift[:, m:m + 1], in0=mean[:, m:m + 1],
                    scalar=nsc[:, m:m + 1], in1=small_params["beta"][:, m:m + 1],
                    op0=OP.mult, op1=OP.add)

            # stage 5: BN + SiLU (sigmoid*identity; AF.Silu NaNs on HW)
            for m in range(2):
                for ct in range(8):
                    ysl = ypre[:, m, ct * 512:(ct + 1) * 512]
                    sg = wp.tile([128, 512], BF16, name="sgt", tag="gt")
                    nc.scalar.activation(out=sg[:], in_=ysl, func=AF.Sigmoid,
                                         bias=shift[:, m:m + 1],
                                         scale=scale[:, m:m + 1])
                    yb = wp.tile([128, 512], BF16, name="ybn", tag="osb")
                    nc.scalar.activation(out=yb[:], in_=ysl, func=AF.Identity,
                                         bias=shift[:, m:m + 1],
                                         scale=scale[:, m:m + 1])
                    nc.vector.tensor_tensor(
                        out=_win(yact[m], (ct * 8 + 2) * P2 + 2, P2),
                        in0=bass.AP(yb[:].tensor, yb[:].offset,
                                    [yb[:].ap[0], [64, 8], [1, 64]]),
                        in1=bass.AP(sg[:].tensor, sg[:].offset,
                                    [sg[:].ap[0], [64, 8], [1, 64]]),
                        op=OP.mult)

            # -------- stages 6/7: depthwise convs, taps split PE/DVE/ACT ----
            wd7_sb = None

            def dw_group(src, dst, wpe_sb, wv_sb, bias, npe, ndv, nact,
                         ksz, dil, PS, m, g):
                """One (m, half) group: PE tap-outer over 4 banks; DVE
                product+add chain at [128,2048]; ACT products merged by DVE;
                DVE partial folded into PSUM via identity matmul."""
                def dxy(t):
                    return (t // ksz) * dil, (t % ksz) * dil

                r0 = g * 32
                accv = ap_.tile([128, 2048], BF16, name="av", tag="accv", bufs=2)
                # DVE product+add chain over its taps
                for ti in range(ndv):
                    dy, dx = dxy(npe + ti)
                    w_ = _win(src[m], (r0 + dy) * PS + dx, PS, nrows=32)
                    sc = wv_sb[:, m, ti:ti + 1]
                    if ti == 0:
                        nc.vector.tensor_scalar(
                            out=accv[:], in0=w_, scalar1=sc,
                            scalar2=None, op0=OP.mult)
                    else:
                        tmp = ap_.tile([128, 2048], BF16, name="tp",
                                       tag="tmp", bufs=2)
                        nc.vector.tensor_scalar(
                            out=tmp[:], in0=w_, scalar1=sc,
                            scalar2=None, op0=OP.mult)
                        nc.vector.tensor_tensor(
                            out=accv[:], in0=accv[:], in1=tmp[:], op=OP.add)
                # ACT products over its taps, DVE adds into accv
                for ti in range(nact):
                    dy, dx = dxy(npe + ndv + ti)
                    w_ = _win(src[m], (r0 + dy) * PS + dx, PS, nrows=32)
                    sc = wv_sb[:, m, ndv + ti:ndv + ti + 1]
                    tmp2 = ap_.tile([128, 2048], BF16, name="tp2",
                                    tag="tmp2", bufs=2)
                    nc.scalar.activation(out=tmp2[:], in_=w_, func=AF.Copy,
                                         scale=sc)
                    eng = nc.gpsimd if ti % 2 else nc.vector
                    eng.tensor_tensor(
                        out=accv[:], in0=accv[:], in1=tmp2[:], op=OP.add)
                # PE: tap-outer over 4 banks
                banks = [pp.tile([128, 512], F32, name="mm", tag="mm")
                         for _ in range(4)]
                for ti in range(npe):
                    dy, dx = dxy(ti)
                    for cb in range(4):
                        ct = g * 4 + cb
                        nc.tensor.matmul(
                            out=banks[cb][:],
                            lhsT=wpe_sb[:, (ti * 2 + m) * 128:
                                        (ti * 2 + m) * 128 + 128],
                            rhs=_win(src[m], (ct * 8 + dy) * PS + dx, PS),
                            start=(ti == 0), stop=False)
                # fold DVE acc via identity matmul, then ACT eviction w/ bias
                for cb in range(4):
                    nc.tensor.matmul(
                        out=banks[cb][:], lhsT=ident_sb[:],
                        rhs=accv[:, cb * 512:(cb + 1) * 512],
                        start=False, stop=True)
                for cb in range(4):
                    ct = g * 4 + cb
                    nc.scalar.activation(
                        out=dst(m, ct), in_=banks[cb][:],
                        func=AF.Identity, bias=bias[:, m:m + 1])

            def dst5(m, ct):
                return _win(z1[m], (ct * 8 + 9) * P9 + 9, P9)

            # dw5: all four (m, g) groups; load dw7 weights during first group
            for mi, (m, g) in enumerate(((0, 0), (0, 1), (1, 0), (1, 1))):
                dw_group(yact, dst5, wd5_sb, w5v_sb, small_params["b5"],
                         PE5, DV5, AC5, 5, 1, P2, m, g)
                if mi == 0:
                    wd7_sb = ap_.tile([128, PE7 * 2 * 128], BF16,
                                      name="wdw2", tag="xoff2")
                    nc.sync.dma_start(out=wd7_sb[:], in_=wd7[:])

            z2 = ap_.tile([128, 2, HW], BF16, name="z2", tag="ypre2")
            gated = ap_.tile([128, 2, HW], BF16, name="gated", tag="ypre")

            def dst7(m, ct):
                return z2[:, m, ct * 512:(ct + 1) * 512]

            def stage89(g):
                # per-ct interleave: gate (GPSIMD) feeds the final conv while
                # the next ct's 1x1 runs
                for cb in range(4):
                    ct = g * 4 + cb
                    for m in range(2):
                        ps = p6.tile([128, 512], F32, name="s8", tag="ptr")
                        for k in range(2):
                            nc.tensor.matmul(
                                out=ps[:],
                                lhsT=wlka1_sb[:, (k * 2 + m) * 128:
                                              (k * 2 + m) * 128 + 128],
                                rhs=z2[:, k, ct * 512:(ct + 1) * 512],
                                start=(k == 0), stop=(k == 1))
                        gt_ = wp.tile([128, 512], BF16, name="gt", tag="gt")
                        nc.scalar.activation(out=gt_[:], in_=ps[:],
                                             func=AF.Identity,
                                             bias=small_params["blka1"][:, m:m + 1])
                        nc.gpsimd.tensor_tensor(
                            out=gated[:, m, ct * 512:(ct + 1) * 512],
                            in0=gt_[:],
                            in1=_win(yact[m], (ct * 8 + 2) * P2 + 2, P2),
                            op=OP.mult)
                    for m in range(2):
                        ps = p6.tile([128, 512], F32, name="s9", tag="ptr")
                        for k in range(2):
                            nc.tensor.matmul(
                                out=ps[:],
                                lhsT=wconv_sb[:, (k * 2 + m) * 128:
                                              (k * 2 + m) * 128 + 128],
                                rhs=gated[:, k, ct * 512:(ct + 1) * 512],
                                start=(k == 0), stop=False)
                        nc.tensor.matmul(
                            out=ps[:], lhsT=ident_sb[:],
                            rhs=_win(xpad_sb[m], (ct * 8 + 1) * P1 + 1, P1),
                            start=False, stop=True)
                        osb = wp.tile([128, 512], F32, name="osb", tag="osb")
                        nc.scalar.activation(out=osb[:], in_=ps[:],
                                             func=AF.Identity,
                                             bias=small_params["bconv"][:, m:m + 1])
                        nc.sync.dma_start(out=out[m][:, ct * 512:(ct + 1) * 512],
                                          in_=osb[:])

            # dw7 g-outer; stage 8/9 of half g interleaves into dw7 g+1
            dw_group(z1, dst7, wd7_sb, w7v_sb, small_params["b7"],
                     PE7, DV7, AC7, 7, 3, P9, 0, 0)
            dw_group(z1, dst7, wd7_sb, w7v_sb, small_params["b7"],
                     PE7, DV7, AC7, 7, 3, P9, 1, 0)
            dw_group(z1, dst7, wd7_sb, w7v_sb, small_params["b7"],
                     PE7, DV7, AC7, 7, 3, P9, 0, 1)
            stage89(0)
            dw_group(z1, dst7, wd7_sb, w7v_sb, small_params["b7"],
                     PE7, DV7, AC7, 7, 3, P9, 1, 1)
            stage89(1)

    nc.compile()
    return nc


_CACHE = {}


def _pack(inputs, n_cores):
    bf = ml_dtypes.bfloat16
    x = np.asarray(inputs["x"], np.float32)
    p_w = np.asarray(inputs["p_w"], np.float32)
    ak_w = np.asarray(inputs["ak_w"], np.float32)

    woff = np.zeros((128, 18, 6), np.float32)
    for k in range(2):
        for t in range(9):
            woff[:, k * 9 + t, :] = p_w[:, k * 128:(k + 1) * 128, t // 3, t % 3].T
    wak = np.zeros((128, 12, 128), np.float32)
    for n in range(3):
        for k in range(2):
            for m in range(2):
                wak[:, n * 4 + k * 2 + m, :] = ak_w[m * 128:(m + 1) * 128,
                                                    k * 128:(k + 1) * 128, n, 0].T

    def diag_flat(w2d, taps):  # taps: list of tap idx -> [128, len*2, 128]
        o = np.zeros((128, len(taps) * 2, 128), np.float32)
        idx = np.arange(128)
        for j, t in enumerate(taps):
            for m in range(2):
                o[idx, j * 2 + m, idx] = w2d[m * 128:(m + 1) * 128, t]
        return o.reshape(128, -1).astype(bf)

    def tap_scalars(w2d, taps):  # -> [128, 2, len] f32
        o = np.zeros((128, 2, len(taps)), np.float32)
        for j, t in enumerate(taps):
            for m in range(2):
                o[:, m, j] = w2d[m * 128:(m + 1) * 128, t]
        return o

    def chunk2(v):
        return np.asarray(v, np.float32).reshape(2, 128).T.copy()

    def onebyone(w):
        w = np.asarray(w, np.float32).reshape(C, C)
        o = np.zeros((128, 4, 128), np.float32)
        for k in range(2):
            for m in range(2):
                o[:, k * 2 + m, :] = w[m * 128:(m + 1) * 128,
                                       k * 128:(k + 1) * 128].T
        return o.reshape(128, -1).astype(bf)

    pp_, blk = np.meshgrid(np.arange(128), np.arange(NBLK), indexing="ij")
    pix = blk * 128 + pp_
    basex = ((pix // 64)[:, :, None] + np.array([0., 0., 1.])[None, None, :])
    basey = ((pix % 64)[:, :, None] + np.array([0., 1., 0.])[None, None, :])

    w5 = np.asarray(inputs["lka0_w"], np.float32).reshape(C, 25)
    w7 = np.asarray(inputs["lkas_w"], np.float32).reshape(C, 49)

    shared = dict(
        basex=basex.astype(np.float32), basey=basey.astype(np.float32),
        woff=woff.reshape(128, 108).astype(bf),
        pb=np.asarray(inputs["p_b"], np.float32).reshape(6, 1),
        wak=wak.reshape(128, -1).astype(bf),
        gamma=chunk2(inputs["ak_gamma"]), beta=chunk2(inputs["ak_beta"]),
        wd5=diag_flat(w5, list(range(PE5))),
        wd7=diag_flat(w7, list(range(PE7))),
        w5v=tap_scalars(w5, list(range(PE5, 25))),
        w7v=tap_scalars(w7, list(range(PE7, 49))),
        b5=chunk2(inputs["lka0_b"]), b7=chunk2(inputs["lkas_b"]),
        wlka1=onebyone(inputs["lka1_w"]), blka1=chunk2(inputs["lka1_b"]),
        wconv=onebyone(inputs["conv_w"]), bconv=chunk2(inputs["conv_b"]),
        ident=np.eye(128, dtype=np.float32).astype(bf),
        ident32=np.eye(128, dtype=np.float32),
    )

    in_maps = []
    for i in range(n_cores):
        xi = x[i].reshape(C, H, W)
        xp1 = np.zeros((C, P1, P1), np.float32)
        xp1[:, 1:65, 1:65] = xi
        m = dict(shared)
        m["xpad"] = xp1.reshape(2, 128, P1 * P1).astype(bf)
        r = np.arange(128)
        f, c = r // 2, np.minimum((r + 1) // 2, 63)
        xf = xi[:, f, :]          # (C, 128, 64)
        xc = xi[:, c, :]
        p4 = np.empty((128, 128, 4, C), np.float32)
        p4[:, :, 0, :] = xf[:, :, f].transpose(1, 2, 0)   # lt (fx, fy)
        p4[:, :, 1, :] = xc[:, :, c].transpose(1, 2, 0)   # rb (cx, cy)
        p4[:, :, 2, :] = xf[:, :, c].transpose(1, 2, 0)   # lb (fx, cy)
        p4[:, :, 3, :] = xc[:, :, f].transpose(1, 2, 0)   # rt (cx, fy)
        m["xpm4"] = p4.reshape(128 * 128, 1024).astype(bf)
        in_maps.append(m)
    return in_maps


def kernel(**inputs) -> np.ndarray:
    n_cores = 8
    if n_cores not in _CACHE:
        _CACHE[n_cores] = build_graph(n_cores)
    nc = _CACHE[n_cores]
    in_maps = _pack(inputs, n_cores)
    res = run_bass_kernel_spmd(nc, in_maps, list(range(n_cores)))
    outs = [np.asarray(res.results[i]["out"], np.float32).reshape(C, H, W)
            for i in range(n_cores)]
    return np.stack(outs).astype(np.float32)
